# revision 32
# baseline (speedup 1.0000x reference)
"""Trainium2 Bass kernel for SAGAN-style self-attention (nn_Attention_full).

Reference computation (B=4, C_IN=128, C_OUT=64, H=W=64, N=4096):
    f = Wf@x+bf; g = Wg@x+bg; h = Wh@x+bh          (1x1 convs, per batch)
    s[n,m] = f[:,n].g[:,m];  beta = softmax_m(s)
    o = beta @ h^T;  out = gamma*(Wa@o^T + ba)

Sharding: 8 cores = (batch b in 0..3) x (query half in 0..1).
Each core handles 2048 queries x 4096 keys of one batch.

Math restructuring (exact):
  * bg shifts every s row by a per-query constant -> softmax-invariant -> dropped.
  * sum_m beta = 1  ->  bh contribution = +bh after normalize -> folded (with ba,
    gamma) into a host-side bias2 = gamma*(Wa@bh + ba).
  * softmax normalization commutes with the channel-mixing Wa matmul -> the
    device returns rows 0..63 = gamma*Wa @ (exp(s) @ h'^T) and row 64 =
    sum_m exp(s); host divides and adds bias2.
  * No max-subtraction: |s| <= ~20 here, exp is fp32-safe, result identical.

Device layout (per core) -- keys-on-partitions everywhere, zero transposes:
  fd [128, 2048] = WfT.T @ xq (+bf), duplicated in both partition halves
  gd [128, 2048] = WgT.T @ xk, key chunks alternating partition halves
  hT [128, 32, 65] slot mi = (xk chunk).T @ (gamma*Wa@Wh).T ; col 64 = ones
  per query-block qb (512):
    sT chunks (row-packed pairs) -> 3-chunk psum tiles
    pT [128, 32, 512] = exp(sT - 12)     (ScalarE, PSUM->SBUF, FD=1536 ops)
    o psum [65, 512] += matmul(lhsT=hT[:,mi,:], rhs=pT[:,mi,:])  over mi
      (rows 0..63 already Wa-projected; row 64 = softmax denominators)
    copy -> DMA -> out [65, 2048]; host divides by row 64 and adds bias2
"""

import os
import sys

for _p in ("/opt/trn_rl_repo", "/root/.axon_site/_ro/trn_rl_repo"):
    if os.path.isdir(_p) and _p not in sys.path:
        sys.path.insert(0, _p)

import numpy as np

import concourse.bass as bass
import concourse.tile as tile
from concourse import bacc, mybir
from concourse.bass import ts
from concourse.bass_utils import run_bass_kernel_spmd

# ---- problem constants (hardcoded per the spec) ----
B, C_IN, C_OUT, H, W = 4, 128, 64, 64, 64
N = H * W            # 4096 keys
NQ = N // 2          # 2048 queries per core
QB = 512             # query block (one PSUM bank of fp32)
NQB = NQ // QB       # 4
MC = 128             # key chunk (PE output partitions)
NMC = N // MC        # 32
CO1 = C_OUT + 1      # 65: value channels + ones column (softmax denominator)

_F32 = mybir.dt.float32
_F32R = mybir.dt.float32r
_FP16 = mybir.dt.float16
_DT_MM = _FP16   # matmul operand dtype (PSUM accumulation is fp32 regardless)
EXP_SHIFT = -12.0  # exp(s + EXP_SHIFT): keeps exp(s) in fp16 range; cancels in
                   # the softmax normalization (both out rows share the scale)


def _slot_to_keychunk(mi):
    # pT/hT slot -> key chunk; slots 2p/2p+1 are the two concurrent QK
    # row-half outputs of pair p (top half / bottom half of gd).
    t, c, h = mi // 8, (mi // 2) % 4, mi % 2
    return 8 * t + 4 * h + c


def _emit(tc):
    nc = tc.nc
    xk = nc.dram_tensor("xk", [C_IN, N], _DT_MM, kind="ExternalInput").ap()
    xq = nc.dram_tensor("xq", [C_IN, NQ], _DT_MM, kind="ExternalInput").ap()
    wfT = nc.dram_tensor("wfT", [C_IN, C_OUT], _DT_MM, kind="ExternalInput").ap()
    wgT = nc.dram_tensor("wgT", [C_IN, C_OUT], _DT_MM, kind="ExternalInput").ap()
    # whT carries the FUSED value+output projection (gamma*Wa@Wh).T so the
    # PV accumulation directly yields the final projected rows (the Wa matmul
    # commutes with the softmax normalization and the key-sum).
    whT = nc.dram_tensor("whT", [C_IN, C_OUT], _DT_MM, kind="ExternalInput").ap()
    bf = nc.dram_tensor("bf", [C_IN, 1], _F32, kind="ExternalInput").ap()
    out = nc.dram_tensor("out", [CO1, NQ], _F32, kind="ExternalOutput").ap()

    from contextlib import ExitStack

    with ExitStack() as ctx:
        consts = ctx.enter_context(tc.tile_pool(name="consts", bufs=1))
        data = ctx.enter_context(tc.tile_pool(name="data", bufs=1))
        pT_pool = ctx.enter_context(tc.tile_pool(name="pT", bufs=2))
        fin_pool = ctx.enter_context(tc.tile_pool(name="fin", bufs=2))
        # 8 PSUM banks: 2x 3-bank QK tiles (exp reads FD=1536 in one op to
        # amortize the ~293ns ACTIVATE overhead) + 1 for the oT accumulator +
        # 1 for prologue/fin.
        ps_s = ctx.enter_context(tc.tile_pool(name="ps_s", bufs=2, space="PSUM"))
        ps_o = ctx.enter_context(tc.tile_pool(name="ps_o", bufs=1, space="PSUM"))
        ps_h = ctx.enter_context(tc.tile_pool(name="ps_h", bufs=1, space="PSUM"))

        Exp = mybir.ActivationFunctionType.Exp
        Ident = mybir.ActivationFunctionType.Identity

        # ---- load constants & inputs (all matmul operands arrive as bf16) ----
        wfT_sb = consts.tile([C_IN, C_OUT], _DT_MM)
        wgT_sb = consts.tile([C_IN, C_OUT], _DT_MM)
        whT_sb = consts.tile([C_IN, C_OUT], _DT_MM)
        bf_sb = consts.tile([C_IN, 1], _F32)
        xk_sb = data.tile([C_IN, N], _DT_MM)
        xq_sb = data.tile([C_IN, NQ], _DT_MM)
        # Transfers spread over four engines' DMA queues (~50 GB/s each),
        # ordered by demand: f0 needs wfT+xq[0:512]; g0 needs wgT+xk[0:1024].
        nc.sync.dma_start(wfT_sb, wfT)
        nc.sync.dma_start(xq_sb[:, ts(0, 512)], xq[:, ts(0, 512)])
        nc.sync.dma_start(wgT_sb, wgT)
        nc.sync.dma_start(bf_sb, bf)
        nc.scalar.dma_start(xk_sb[:, ts(0, 512)], xk[:, ts(0, 512)])
        nc.gpsimd.dma_start(xk_sb[:, ts(1, 512)], xk[:, ts(1, 512)])
        nc.gpsimd.dma_start(whT_sb, whT)
        nc.sync.dma_start(xq_sb[:, ts(1, 512)], xq[:, ts(1, 512)])
        nc.scalar.dma_start(xk_sb[:, ts(1, 1024)], xk[:, ts(1, 1024)])
        nc.gpsimd.dma_start(xk_sb[:, ts(2, 1024)], xk[:, ts(2, 1024)])
        nc.gpsimd.dma_start(xk_sb[:, ts(3, 1024)], xk[:, ts(3, 1024)])
        nc.sync.dma_start(xq_sb[:, ts(1, 1024)], xq[:, ts(1, 1024)])

        # ---- PE warm-up burst ----
        # The HAM clock gate starts at K=4/8 (1.2 GHz) and needs ~3.4us of
        # sustained PE activity to release. Burn dummy matmuls on a zeroed
        # scratch tile while the input DMAs land so the real work runs warm.
        warm_sb = consts.tile([C_IN, 640], _DT_MM)
        nc.vector.memset(warm_sb, 0.0)
        wps = ps_s.tile([MC, 2, QB], _F32, tag="s")
        for _ in range(12):
            nc.tensor.matmul(wps[:, 0, :], warm_sb[:, 0:MC],
                             warm_sb[:, MC:MC + QB], start=True, stop=True)

        # ---- projections ----
        # fd: f duplicated into both partition halves (QK row-packing rhs);
        # built by two column-tiled matmuls into one [128, 512] psum.
        # Only block 0 of f/g is built up front; the rest are emitted as
        # filler inside the first query block's loop (demand-ordered), so
        # the first exp fires as early as possible.
        fd_sb = data.tile([C_IN, NQ], _DT_MM)
        gd_sb = data.tile([C_IN, N // 2], _DT_MM)
        hT_sb = data.tile([C_IN, NMC, CO1], _DT_MM)

        ones_sb = consts.tile([C_IN, NMC, 1], _F32)
        nc.vector.memset(ones_sb, 1.0)
        nc.vector.tensor_copy(hT_sb[:, :, C_OUT:CO1], ones_sb)
        shift_sb = consts.tile([MC, 1], _F32)
        nc.vector.memset(shift_sb, EXP_SHIFT)

        def build_f(j, pool=None):
            ps = (pool or ps_h).tile([C_IN, 512], _F32,
                                     tag="o" if pool is ps_o else "h")
            rhs = xq_sb[:, ts(j, 512)]
            nc.tensor.matmul(ps[0:C_OUT, :], wfT_sb, rhs, start=True,
                             stop=True, tile_position=(0, 0))
            nc.tensor.matmul(ps[C_OUT:C_IN, :], wfT_sb, rhs, start=True,
                             stop=True, tile_position=(0, 64))
            nc.vector.tensor_scalar_add(fd_sb[:, ts(j, 512)], ps, bf_sb)

        def build_g(t):
            # key block pair (1024t..+512 -> top half, +512..+1024 -> bottom)
            ps = ps_h.tile([C_IN, 512], _F32, tag="h")
            nc.tensor.matmul(ps[0:C_OUT, :], wgT_sb,
                             xk_sb[:, 1024 * t:1024 * t + 512], start=True,
                             stop=True, tile_position=(0, 0))
            nc.tensor.matmul(ps[C_OUT:C_IN, :], wgT_sb,
                             xk_sb[:, 1024 * t + 512:1024 * t + 1024],
                             start=True, stop=True, tile_position=(0, 64))
            nc.vector.tensor_copy(gd_sb[:, ts(t, 512)], ps)

        def build_hT_group(g):
            # 8 hT slots share one psum bank: [128 keys, 64 ch] per slot =
            # xk_chunk.T @ WhT, then a single strided copy; col 64 stays 1.0.
            # Deprioritized: QK/exp must win the PE; the lagged PV consumers
            # leave plenty of slack.
            with tc.high_priority(offset=-64):
                ps = ps_h.tile([MC, 8, C_OUT], _F32, tag="h")
                for i in range(8):
                    kc = _slot_to_keychunk(8 * g + i)
                    nc.tensor.matmul(ps[:, i, :], xk_sb[:, ts(kc, MC)],
                                     whT_sb, start=True, stop=True)
                nc.vector.tensor_copy(hT_sb[:, 8 * g:8 * g + 8, 0:C_OUT], ps)

        def keep_warm(n):
            # PE_HAM re-throttles when the activity window sees mostly-idle;
            # pad the DMA-wait holes in the startup chain with dummy matmuls
            # so the whole first query block runs at 2.4 GHz.
            for _ in range(n):
                nc.tensor.matmul(wps[:, 0, :], warm_sb[:, 0:MC],
                                 warm_sb[:, MC:MC + QB], start=True, stop=True)

        build_f(0, pool=ps_o)
        keep_warm(3)
        build_g(0)
        keep_warm(3)
        # demand-ordered prologue filler inside qb0: chunk index -> builders
        # emission-order deadlines: gd block t before QK chunk 8t; hT group g
        # before the (2-tile-lagged) PV of slot 8g
        qb0_filler = {
            1: [lambda: build_g(1)],
            6: [lambda: build_hT_group(0)],
            7: [lambda: build_g(2)],
            12: [lambda: build_hT_group(1)],
            15: [lambda: build_g(3)],
            18: [lambda: build_hT_group(2)],
            24: [lambda: build_hT_group(3)],
            27: [lambda: build_f(1)],
            29: [lambda: build_f(2)],
            31: [lambda: build_f(3)],
        }

        # ---- attention main loop ----
        # Chunk ci = pT/hT slot: even ci -> row-half A (PE rows 0-63), odd ->
        # row-half B (rows 64-127); A/B of a pair run concurrently via
        # tile_position row packing. QK chunks land in 3-chunk psum tiles;
        # one exp (FD=1536) per filled tile, then that tile's PV accumulation
        # matmuls -- PV interleaves with QK so the PE never sits on a serial
        # PV tail after the last exp.
        for qb in range(NQB):
            fqA = fd_sb[0:C_OUT, ts(qb, QB)]
            fqB = fd_sb[C_OUT:C_IN, ts(qb, QB)]
            pT = pT_pool.tile([MC, NMC, QB], _DT_MM)
            # alternate the accumulator's bank per qb so the next block's
            # start=True never waits on this block's drain copy
            o_pool, o_tag = (ps_o, "o") if qb % 2 == 0 else (ps_h, "h")
            o_ps = o_pool.tile([CO1, QB], _F32, tag=o_tag)

            def flush_pv(tiles):
                with tc.high_priority(offset=-64):
                    for b0, e0 in tiles:
                        for mi in range(b0, e0 + 1):
                            nc.tensor.matmul(o_ps, hT_sb[:, mi, :],
                                             pT[:, mi, :],
                                             start=(mi == 0),
                                             stop=(mi == NMC - 1),
                                             skip_group_check=True)

            ps, base, pending = None, 0, []
            for ci in range(NMC):
                if qb == 0:
                    for fn in qb0_filler.get(ci, ()):
                        fn()
                p, half = ci // 2, ci % 2
                gcol = 512 * (p // 4) + 128 * (p % 4)
                if ps is None:
                    width = min(3, NMC - ci)
                    ps = ps_s.tile([MC, width, QB], _F32, tag="s")
                    base = ci
                g_half = gd_sb[0:C_OUT, gcol:gcol + 128] if half == 0 else \
                    gd_sb[C_OUT:C_IN, gcol:gcol + 128]
                nc.tensor.matmul(ps[:, ci - base, :], g_half,
                                 fqA if half == 0 else fqB,
                                 start=True, stop=True,
                                 tile_position=(0, 0) if half == 0 else (64, 0))
                if ci - base == width - 1:
                    nc.scalar.activation(pT[:, base:ci + 1, :], ps, Exp,
                                         bias=shift_sb)
                    pending.append((base, ci))
                    lag = 1 if qb == NQB - 1 else 2
                    if len(pending) > lag:
                        flush_pv([pending.pop(0)])
                    ps = None
            flush_pv(pending)

            fin_sb = fin_pool.tile([CO1, QB], _F32, tag="fin")
            for h2 in range(2):
                cols = slice(256 * h2, 256 * h2 + 256)
                nc.vector.tensor_copy(fin_sb[:, cols], o_ps[:, cols])
                nc.sync.dma_start(
                    out[:, qb * QB + 256 * h2:qb * QB + 256 * h2 + 256],
                    fin_sb[:, cols])


_NC_CACHE = {}


def _get_nc():
    if "nc" not in _NC_CACHE:
        nc = bacc.Bacc("TRN2", target_bir_lowering=False, debug=False)
        with tile.TileContext(nc) as tc:
            _emit(tc)
        nc.compile()
        _NC_CACHE["nc"] = nc
    return _NC_CACHE["nc"]


def _prepare(inputs):
    x = np.asarray(inputs["x"], dtype=np.float32)
    Wf = np.asarray(inputs["Wf"], dtype=np.float32)
    bf = np.asarray(inputs["bf"], dtype=np.float32)
    Wg = np.asarray(inputs["Wg"], dtype=np.float32)
    Wh = np.asarray(inputs["Wh"], dtype=np.float32)
    bh = np.asarray(inputs["bh"], dtype=np.float32)
    Wa = np.asarray(inputs["Wa"], dtype=np.float32)
    ba = np.asarray(inputs["ba"], dtype=np.float32)
    gamma = float(np.asarray(inputs["gamma"]).reshape(-1)[0])

    bft = np.float16
    xf = np.ascontiguousarray(x.reshape(B, C_IN, N)).astype(bft)
    wfT = np.ascontiguousarray(Wf.T).astype(bft)
    wgT = np.ascontiguousarray(Wg.T).astype(bft)
    # fused value+output projection: PV then directly yields gamma*Wa@(p@h'^T)
    whaT = np.ascontiguousarray((gamma * Wa @ Wh).T).astype(bft)
    bf2 = np.ascontiguousarray(
        np.concatenate([bf, bf]).reshape(C_IN, 1).astype(np.float32))
    bias2 = gamma * (Wa @ bh + ba)  # folded bh/ba/gamma bias, added on host

    in_maps = []
    for core in range(8):
        b, half = core // 2, core % 2
        in_maps.append({
            "xk": xf[b],
            "xq": np.ascontiguousarray(xf[b][:, half * NQ:(half + 1) * NQ]),
            "wfT": wfT, "wgT": wgT, "whT": whaT, "bf": bf2,
        })

    def post(results):
        O = np.empty((B, C_OUT, N), dtype=np.float32)
        for core in range(8):
            b, half = core // 2, core % 2
            r = results[core]["out"]
            O[b][:, half * NQ:(half + 1) * NQ] = (
                r[:C_OUT] / r[C_OUT:CO1] + bias2[:, None])
        return O.reshape(B, C_OUT, H, W)

    return in_maps, post


def kernel(**inputs):
    in_maps, post = _prepare(inputs)
    res = run_bass_kernel_spmd(_get_nc(), in_maps, core_ids=list(range(8)))
    return post(res.results)


def kernel_traced(**inputs):
    """Like kernel() but with NTFF profiling; returns (output, BassKernelResults)."""
    in_maps, post = _prepare(inputs)
    res = run_bass_kernel_spmd(_get_nc(), in_maps, core_ids=list(range(8)),
                               trace=True)
    return post(res.results), res


# revision 38
# speedup vs baseline: 1.0212x; 1.0212x over previous
"""Trainium2 Bass kernel for SAGAN-style self-attention (nn_Attention_full).

Reference computation (B=4, C_IN=128, C_OUT=64, H=W=64, N=4096):
    f = Wf@x+bf; g = Wg@x+bg; h = Wh@x+bh          (1x1 convs, per batch)
    s[n,m] = f[:,n].g[:,m];  beta = softmax_m(s)
    o = beta @ h^T;  out = gamma*(Wa@o^T + ba)

Sharding: 8 cores = (batch b in 0..3) x (query half in 0..1).
Each core handles 2048 queries x 4096 keys of one batch.

Math restructuring (exact):
  * bg shifts every s row by a per-query constant -> softmax-invariant -> dropped.
  * sum_m beta = 1  ->  bh contribution = +bh after normalize -> folded (with ba,
    gamma) into a host-side bias2 = gamma*(Wa@bh + ba).
  * softmax normalization commutes with the channel-mixing Wa matmul -> the
    device returns rows 0..63 = gamma*Wa @ (exp(s) @ h'^T) and row 64 =
    sum_m exp(s); host divides and adds bias2.
  * No max-subtraction: |s| <= ~20 here, exp is fp32-safe, result identical.

Device layout (per core) -- keys-on-partitions everywhere, zero transposes:
  fd [128, 2048] = WfT.T @ xq (+bf), duplicated in both partition halves
  gd [128, 2048] = WgT.T @ xk, key chunks alternating partition halves
  hT [128, 32, 65] slot mi = (xk chunk).T @ (gamma*Wa@Wh).T ; col 64 = ones
  per query-block qb (512):
    sT chunks (row-packed pairs) -> 3-chunk psum tiles
    pT [128, 32, 512] = exp(sT - 12)     (ScalarE, PSUM->SBUF, FD=1536 ops)
    o psum [65, 512] += matmul(lhsT=hT[:,mi,:], rhs=pT[:,mi,:])  over mi
      (rows 0..63 already Wa-projected; row 64 = softmax denominators)
    copy -> DMA -> out [65, 2048]; host divides by row 64 and adds bias2
"""

import os
import sys

for _p in ("/opt/trn_rl_repo", "/root/.axon_site/_ro/trn_rl_repo"):
    if os.path.isdir(_p) and _p not in sys.path:
        sys.path.insert(0, _p)

import numpy as np

import concourse.bass as bass
import concourse.tile as tile
from concourse import bacc, mybir
from concourse.bass import ts
from concourse.bass_utils import run_bass_kernel_spmd

# ---- problem constants (hardcoded per the spec) ----
B, C_IN, C_OUT, H, W = 4, 128, 64, 64, 64
N = H * W            # 4096 keys
NQ = N // 2          # 2048 queries per core
QB = 512             # query block (one PSUM bank of fp32)
NQB = NQ // QB       # 4
MC = 128             # key chunk (PE output partitions)
NMC = N // MC        # 32
CO1 = C_OUT + 1      # 65: value channels + ones column (softmax denominator)

_F32 = mybir.dt.float32
_F32R = mybir.dt.float32r
_FP16 = mybir.dt.float16
_DT_MM = _FP16   # matmul operand dtype (PSUM accumulation is fp32 regardless)
EXP_SHIFT = -12.0  # exp(s + EXP_SHIFT): keeps exp(s) in fp16 range; cancels in
                   # the softmax normalization (both out rows share the scale)


def _slot_to_keychunk(mi):
    # pT/hT slot -> key chunk; slots 2p/2p+1 are the two concurrent QK
    # row-half outputs of pair p (top half / bottom half of gd).
    t, c, h = mi // 8, (mi // 2) % 4, mi % 2
    return 8 * t + 4 * h + c


def _emit(tc):
    nc = tc.nc
    xk = nc.dram_tensor("xk", [C_IN, N], _DT_MM, kind="ExternalInput").ap()
    xq = nc.dram_tensor("xq", [C_IN, NQ], _DT_MM, kind="ExternalInput").ap()
    wfT = nc.dram_tensor("wfT", [C_IN, C_OUT], _DT_MM, kind="ExternalInput").ap()
    wgT = nc.dram_tensor("wgT", [C_IN, C_OUT], _DT_MM, kind="ExternalInput").ap()
    # whT carries the FUSED value+output projection (gamma*Wa@Wh).T so the
    # PV accumulation directly yields the final projected rows (the Wa matmul
    # commutes with the softmax normalization and the key-sum).
    whT = nc.dram_tensor("whT", [C_IN, C_OUT], _DT_MM, kind="ExternalInput").ap()
    bf = nc.dram_tensor("bf", [C_IN, 1], _F32, kind="ExternalInput").ap()
    out = nc.dram_tensor("out", [CO1, NQ], _F32, kind="ExternalOutput").ap()

    from contextlib import ExitStack

    with ExitStack() as ctx:
        consts = ctx.enter_context(tc.tile_pool(name="consts", bufs=1))
        data = ctx.enter_context(tc.tile_pool(name="data", bufs=1))
        pT_pool = ctx.enter_context(tc.tile_pool(name="pT", bufs=2))
        fin_pool = ctx.enter_context(tc.tile_pool(name="fin", bufs=2))
        # 8 PSUM banks: 2x 3-bank QK tiles (exp reads FD=1536 in one op to
        # amortize the ~293ns ACTIVATE overhead) + 1 for the oT accumulator +
        # 1 for prologue/fin.
        ps_s = ctx.enter_context(tc.tile_pool(name="ps_s", bufs=2, space="PSUM"))
        ps_o = ctx.enter_context(tc.tile_pool(name="ps_o", bufs=1, space="PSUM"))
        ps_h = ctx.enter_context(tc.tile_pool(name="ps_h", bufs=1, space="PSUM"))

        Exp = mybir.ActivationFunctionType.Exp
        Ident = mybir.ActivationFunctionType.Identity

        # ---- load constants & inputs (all matmul operands arrive as bf16) ----
        wfT_sb = consts.tile([C_IN, C_OUT], _DT_MM)
        wgT_sb = consts.tile([C_IN, C_OUT], _DT_MM)
        whT_sb = consts.tile([C_IN, C_OUT], _DT_MM)
        bf_sb = consts.tile([C_IN, 1], _F32)
        xk_sb = data.tile([C_IN, N], _DT_MM)
        xq_sb = data.tile([C_IN, NQ], _DT_MM)
        # Transfers spread over four engines' DMA queues (~50 GB/s each),
        # ordered by demand: f0 needs wfT+xq[0:512]; g0 needs wgT+xk[0:1024].
        nc.sync.dma_start(wfT_sb, wfT)
        nc.sync.dma_start(xq_sb[:, ts(0, 512)], xq[:, ts(0, 512)])
        nc.sync.dma_start(bf_sb, bf)
        nc.scalar.dma_start(wgT_sb, wgT)
        nc.scalar.dma_start(xk_sb[:, ts(0, 512)], xk[:, ts(0, 512)])
        nc.gpsimd.dma_start(xk_sb[:, ts(1, 512)], xk[:, ts(1, 512)])
        nc.gpsimd.dma_start(whT_sb, whT)
        nc.sync.dma_start(xq_sb[:, ts(1, 512)], xq[:, ts(1, 512)])
        nc.scalar.dma_start(xk_sb[:, ts(1, 1024)], xk[:, ts(1, 1024)])
        nc.gpsimd.dma_start(xk_sb[:, ts(2, 1024)], xk[:, ts(2, 1024)])
        nc.gpsimd.dma_start(xk_sb[:, ts(3, 1024)], xk[:, ts(3, 1024)])
        nc.sync.dma_start(xq_sb[:, ts(1, 1024)], xq[:, ts(1, 1024)])

        # ---- PE warm-up burst ----
        # The HAM clock gate starts at K=4/8 (1.2 GHz) and needs ~3.4us of
        # sustained PE activity to release. Burn dummy matmuls on a zeroed
        # scratch tile while the input DMAs land so the real work runs warm.
        warm_sb = consts.tile([C_IN, 640], _DT_MM)
        nc.vector.memset(warm_sb, 0.0)
        wps = ps_s.tile([MC, 2, QB], _F32, tag="s")
        for _ in range(12):
            nc.tensor.matmul(wps[:, 0, :], warm_sb[:, 0:MC],
                             warm_sb[:, MC:MC + QB], start=True, stop=True)

        # ---- projections ----
        # fd: f duplicated into both partition halves (QK row-packing rhs);
        # built by two column-tiled matmuls into one [128, 512] psum.
        # Only block 0 of f/g is built up front; the rest are emitted as
        # filler inside the first query block's loop (demand-ordered), so
        # the first exp fires as early as possible.
        fd_sb = data.tile([C_IN, NQ], _DT_MM)
        gd_sb = data.tile([C_IN, N // 2], _DT_MM)
        hT_sb = data.tile([C_IN, NMC, CO1], _DT_MM)

        ones_sb = consts.tile([C_IN, NMC, 1], _F32)
        nc.vector.memset(ones_sb, 1.0)
        nc.vector.tensor_copy(hT_sb[:, :, C_OUT:CO1], ones_sb)
        shift_sb = consts.tile([MC, 1], _F32)
        nc.vector.memset(shift_sb, EXP_SHIFT)

        def build_f(j, pool=None):
            ps = (pool or ps_h).tile([C_IN, 512], _F32,
                                     tag="o" if pool is ps_o else "h")
            rhs = xq_sb[:, ts(j, 512)]
            nc.tensor.matmul(ps[0:C_OUT, :], wfT_sb, rhs, start=True,
                             stop=True, tile_position=(0, 0))
            nc.tensor.matmul(ps[C_OUT:C_IN, :], wfT_sb, rhs, start=True,
                             stop=True, tile_position=(0, 64))
            nc.vector.tensor_scalar_add(fd_sb[:, ts(j, 512)], ps, bf_sb)

        def build_g(t):
            # key block pair (1024t..+512 -> top half, +512..+1024 -> bottom)
            ps = ps_h.tile([C_IN, 512], _F32, tag="h")
            nc.tensor.matmul(ps[0:C_OUT, :], wgT_sb,
                             xk_sb[:, 1024 * t:1024 * t + 512], start=True,
                             stop=True, tile_position=(0, 0))
            nc.tensor.matmul(ps[C_OUT:C_IN, :], wgT_sb,
                             xk_sb[:, 1024 * t + 512:1024 * t + 1024],
                             start=True, stop=True, tile_position=(0, 64))
            nc.vector.tensor_copy(gd_sb[:, ts(t, 512)], ps)

        def build_hT_group(g):
            # 8 hT slots share one psum bank: [128 keys, 64 ch] per slot =
            # xk_chunk.T @ WhT, then a single strided copy; col 64 stays 1.0.
            # Deprioritized: QK/exp must win the PE; the lagged PV consumers
            # leave plenty of slack.
            with tc.high_priority(offset=-64):
                ps = ps_h.tile([MC, 8, C_OUT], _F32, tag="h")
                for i in range(8):
                    kc = _slot_to_keychunk(8 * g + i)
                    nc.tensor.matmul(ps[:, i, :], xk_sb[:, ts(kc, MC)],
                                     whT_sb, start=True, stop=True)
                nc.vector.tensor_copy(hT_sb[:, 8 * g:8 * g + 8, 0:C_OUT], ps)

        build_f(0, pool=ps_o)
        build_g(0)
        # demand-ordered prologue filler inside qb0: chunk index -> builders
        # emission-order deadlines: gd block t before QK chunk 8t; hT group g
        # before the (2-tile-lagged) PV of slot 8g
        qb0_filler = {
            1: [lambda: build_g(1)],
            6: [lambda: build_hT_group(0)],
            7: [lambda: build_g(2)],
            12: [lambda: build_hT_group(1)],
            15: [lambda: build_g(3)],
            18: [lambda: build_hT_group(2)],
            24: [lambda: build_hT_group(3)],
            27: [lambda: build_f(1)],
            29: [lambda: build_f(2)],
            31: [lambda: build_f(3)],
        }

        # ---- attention main loop ----
        # Chunk ci = pT/hT slot: even ci -> row-half A (PE rows 0-63), odd ->
        # row-half B (rows 64-127); A/B of a pair run concurrently via
        # tile_position row packing. QK chunks land in 3-chunk psum tiles;
        # one exp (FD=1536) per filled tile, then that tile's PV accumulation
        # matmuls -- PV interleaves with QK so the PE never sits on a serial
        # PV tail after the last exp.
        for qb in range(NQB):
            fqA = fd_sb[0:C_OUT, ts(qb, QB)]
            fqB = fd_sb[C_OUT:C_IN, ts(qb, QB)]
            pT = pT_pool.tile([MC, NMC, QB], _DT_MM)
            # alternate the accumulator's bank per qb so the next block's
            # start=True never waits on this block's drain copy
            o_pool, o_tag = (ps_o, "o") if qb % 2 == 0 else (ps_h, "h")
            o_ps = o_pool.tile([CO1, QB], _F32, tag=o_tag)

            def flush_pv(tiles):
                with tc.high_priority(offset=-64):
                    for b0, e0 in tiles:
                        for mi in range(b0, e0 + 1):
                            nc.tensor.matmul(o_ps, hT_sb[:, mi, :],
                                             pT[:, mi, :],
                                             start=(mi == 0),
                                             stop=(mi == NMC - 1),
                                             skip_group_check=True)

            ps, base, pending = None, 0, []
            for ci in range(NMC):
                if qb == 0:
                    for fn in qb0_filler.get(ci, ()):
                        fn()
                p, half = ci // 2, ci % 2
                gcol = 512 * (p // 4) + 128 * (p % 4)
                if ps is None:
                    width = min(3, NMC - ci)
                    ps = ps_s.tile([MC, width, QB], _F32, tag="s")
                    base = ci
                g_half = gd_sb[0:C_OUT, gcol:gcol + 128] if half == 0 else \
                    gd_sb[C_OUT:C_IN, gcol:gcol + 128]
                nc.tensor.matmul(ps[:, ci - base, :], g_half,
                                 fqA if half == 0 else fqB,
                                 start=True, stop=True,
                                 tile_position=(0, 0) if half == 0 else (64, 0))
                if ci - base == width - 1:
                    nc.scalar.activation(pT[:, base:ci + 1, :], ps, Exp,
                                         bias=shift_sb)
                    pending.append((base, ci))
                    lag = 1 if qb == NQB - 1 else 2
                    if len(pending) > lag:
                        flush_pv([pending.pop(0)])
                    ps = None
            flush_pv(pending)

            fin_sb = fin_pool.tile([CO1, QB], _F32, tag="fin")
            if qb == NQB - 1:
                # split the drain so DMA of the first half overlaps the copy
                # of the second (tail is latency-critical on the last block)
                for h2 in range(2):
                    cols = slice(256 * h2, 256 * h2 + 256)
                    nc.vector.tensor_copy(fin_sb[:, cols], o_ps[:, cols])
                    eng = nc.sync if h2 == 0 else nc.scalar
                    eng.dma_start(
                        out[:, qb * QB + 256 * h2:qb * QB + 256 * h2 + 256],
                        fin_sb[:, cols])
            else:
                nc.vector.tensor_copy(fin_sb, o_ps)
                nc.sync.dma_start(out[:, ts(qb, QB)], fin_sb)


_NC_CACHE = {}


def _get_nc():
    if "nc" not in _NC_CACHE:
        nc = bacc.Bacc("TRN2", target_bir_lowering=False, debug=False)
        with tile.TileContext(nc) as tc:
            _emit(tc)
        nc.compile()
        _NC_CACHE["nc"] = nc
    return _NC_CACHE["nc"]


def _prepare(inputs):
    x = np.asarray(inputs["x"], dtype=np.float32)
    Wf = np.asarray(inputs["Wf"], dtype=np.float32)
    bf = np.asarray(inputs["bf"], dtype=np.float32)
    Wg = np.asarray(inputs["Wg"], dtype=np.float32)
    Wh = np.asarray(inputs["Wh"], dtype=np.float32)
    bh = np.asarray(inputs["bh"], dtype=np.float32)
    Wa = np.asarray(inputs["Wa"], dtype=np.float32)
    ba = np.asarray(inputs["ba"], dtype=np.float32)
    gamma = float(np.asarray(inputs["gamma"]).reshape(-1)[0])

    bft = np.float16
    xf = np.ascontiguousarray(x.reshape(B, C_IN, N)).astype(bft)
    wfT = np.ascontiguousarray(Wf.T).astype(bft)
    wgT = np.ascontiguousarray(Wg.T).astype(bft)
    # fused value+output projection: PV then directly yields gamma*Wa@(p@h'^T)
    whaT = np.ascontiguousarray((gamma * Wa @ Wh).T).astype(bft)
    bf2 = np.ascontiguousarray(
        np.concatenate([bf, bf]).reshape(C_IN, 1).astype(np.float32))
    bias2 = gamma * (Wa @ bh + ba)  # folded bh/ba/gamma bias, added on host

    in_maps = []
    for core in range(8):
        b, half = core // 2, core % 2
        in_maps.append({
            "xk": xf[b],
            "xq": np.ascontiguousarray(xf[b][:, half * NQ:(half + 1) * NQ]),
            "wfT": wfT, "wgT": wgT, "whT": whaT, "bf": bf2,
        })

    def post(results):
        O = np.empty((B, C_OUT, N), dtype=np.float32)
        for core in range(8):
            b, half = core // 2, core % 2
            r = results[core]["out"]
            O[b][:, half * NQ:(half + 1) * NQ] = (
                r[:C_OUT] / r[C_OUT:CO1] + bias2[:, None])
        return O.reshape(B, C_OUT, H, W)

    return in_maps, post


def kernel(**inputs):
    in_maps, post = _prepare(inputs)
    res = run_bass_kernel_spmd(_get_nc(), in_maps, core_ids=list(range(8)))
    return post(res.results)


def kernel_traced(**inputs):
    """Like kernel() but with NTFF profiling; returns (output, BassKernelResults)."""
    in_maps, post = _prepare(inputs)
    res = run_bass_kernel_spmd(_get_nc(), in_maps, core_ids=list(range(8)),
                               trace=True)
    return post(res.results), res
